# revision 6
# baseline (speedup 1.0000x reference)
"""CRF token-mean NLL on 8 Trainium2 NeuronCores — time-segmented forward
algorithm with warmup-seeded parallel chains.

Math
----
out = sum_b(llh_b / labels_b) / count,  llh_b = den_b - num_b.
num (gold path score): host gather (numpy f64) — cheap, O(B*S).

den_b = logZ_b at tail_b via the forward algorithm in probability space:
x_j = softmax(em_j) (start folded into x_0), v_j = x_j . (E^T v_{j-1}),
logZ_j = log(w . v_j) + cumsum(a)_j, with a_j the softmax log-shifts and
w = exp(end).

Key idea: the recursion's DIRECTION forgets its initial condition at the
CRF mixing rate (a few steps here), only the log-MAGNITUDE accumulates.
So split the S=1024 steps into M=24 segments; each segment's chain is
seeded at (boundary - W) with the x-tile there and warmed up W=8 steps.
log Z telescopes through per-step captures q_j = w . v_j:
  logZ_tail = lq[0, span0] + sum_{0<s<s*} (lq[s, W+span_s] - lq[s, W])
              + lq[s*, tail-t_s*] - lq[s*, W] + big_a[tail]
(segment 0 is seeded exactly with v_0 = x_0, so its captures are
absolute).  Validated to rel err ~3e-8 vs f64.

Magnitude control: a constant 2^0.8125 per-step factor folded into the
host-prepped x stream keeps bf16 state magnitudes within 2^+-25 over the
51-step chains — no on-device renormalization at all.

Device structure (per core, 128 seqs x 24 segments = 3072 chains):
- State tiles [99, C]: 3 chain-blocks of 32 tags (partitions 0..95)
  + 3 capture rows (96..98).  Column c, block k = chain k*C+c.
- ONE stationary [96, 128] = blockdiag(E,E,E) plus capture columns
  (w replicated per block) padded to 128 for fast weight load; loaded
  effectively once.  Each matmul computes the state update AND
  q = w.v as 3 extra output partitions.
- Column groups (independent rings, staggered chains): per step-row and
  group, PE matmul -> elementwise x-multiply.  Group paths alternate:
  "act" = ScalarE psum->SBUF bf16 copy then VectorE 2x-mode multiply,
  "direct" = VectorE 1x multiply straight from PSUM — balancing ScalarE
  and VectorE load.  The x tiles carry ones in rows 96-98 so captures
  pass through the multiply into the state rings, DMA'd out in batches.
"""

import numpy as np

B, S, T = 1024, 1024, 32
NCORES = 8
SEQ_PER_CORE = 128
M = 24                      # time segments
W = 8                       # warmup steps per seeded chain
NCH = 3                     # chain blocks per tile
C = SEQ_PER_CORE * M // NCH  # 1024 columns
PATHS = ["act", "direct"]   # per-group elementwise path
NG = len(PATHS)
CG = C // NG                # columns per group
P = NCH * T + NCH           # 99 live partitions (96 state + 3 capture)
PFULL = 128                 # padded matmul output partitions
RD = 16                     # ring depth (slots)
CHX = 4                     # x-tiles per streamed chunk
CLOG2 = 0.8125              # per-step 2^CLOG2 folded into x
A_BOUND = [round(S * s / M) for s in range(M + 1)]
SPANS = [A_BOUND[s + 1] - A_BOUND[s] for s in range(M)]
T_START = [0] + [A_BOUND[s] - W for s in range(1, M)]
NSTEP = max(SPANS) + W + 1  # 52 steps: captures q_0..q_{NSTEP-1}

_PROG_CACHE = {}
TRACE = False
LAST_RESULTS = None


def _build_program():
    import concourse.bacc as bacc
    import concourse.mybir as mybir
    from concourse import tile

    f32 = mybir.dt.float32
    bf16 = mybir.dt.bfloat16

    nc = bacc.Bacc("TRN2", target_bir_lowering=False, debug=False,
                   enable_asserts=False, num_devices=NCORES)

    # x stream: tile i at cols [i*C, (i+1)*C); tile 0 is the seed
    xseq_dram = nc.dram_tensor("xseq", [P, (NSTEP + 1) * C], bf16,
                               kind="ExternalInput")
    wcaug_dram = nc.dram_tensor("wcaug", [NCH * T, PFULL], bf16,
                                kind="ExternalInput")
    qcap_dram = [nc.dram_tensor(f"qcap{g}", [NCH, NSTEP * CG], bf16,
                                kind="ExternalOutput") for g in range(NG)]

    nchunks = (NSTEP + CHX - 1) // CHX  # chunks cover tiles 1..NSTEP

    with tile.TileContext(nc) as tc:
        with (
            tc.tile_pool(name="const", bufs=1) as constp,
            tc.tile_pool(name="ringp", bufs=1) as ringp,
            tc.tile_pool(name="xs", bufs=3) as xp,
            tc.tile_pool(name="ct", bufs=4) as ctp,
            tc.tile_pool(name="ps", bufs=4, space="PSUM") as psp,
        ):
            wcaug = constp.tile([NCH * T, PFULL], bf16)
            nc.sync.dma_start(wcaug[:], wcaug_dram[:])

            rings = [ringp.tile([P, RD * CG], bf16, name=f"ring{g}")
                     for g in range(NG)]
            for g in range(NG):
                nc.gpsimd.dma_start(rings[g][:, 0:CG],
                                    xseq_dram[:, g * CG:(g + 1) * CG])

            xtiles = {}

            def ensure_chunk(cid):
                if cid in xtiles or cid >= nchunks:
                    return
                lo = (1 + cid * CHX) * C
                n = min(CHX * C, (NSTEP + 1) * C - lo)
                tf = xp.tile([P, CHX * C], bf16, tag="xc", name="xc")
                half = n // 2
                eng = nc.sync if cid % 2 == 0 else nc.gpsimd
                eng.dma_start(tf[:, 0:half], xseq_dram[:, lo:lo + half])
                eng.dma_start(tf[:, half:n], xseq_dram[:, lo + half:lo + n])
                xtiles[cid] = tf

            ensure_chunk(0)
            ensure_chunk(1)
            ensure_chunk(2)

            def slot(i):
                return (i % RD) * CG

            last_dumped = 0
            for i in range(1, NSTEP + 1):
                cid = (i - 1) // CHX
                off = ((i - 1) % CHX) * C
                if (i - 1) % CHX == 0:
                    ensure_chunk(cid + 3)
                xt = xtiles[cid]
                for g in range(NG):
                    ring = rings[g]
                    xsl = xt[:, off + g * CG:off + (g + 1) * CG]
                    ps = psp.tile([PFULL, CG], f32, tag=f"ps{g}",
                                  name=f"ps{g}")
                    nc.tensor.matmul(
                        ps[:], wcaug[:],
                        ring[0:NCH * T, slot(i - 1):slot(i - 1) + CG])
                    dst = ring[:, slot(i):slot(i) + CG]
                    if PATHS[g] == "act":
                        ct = ctp.tile([P, CG], bf16, tag=f"ct{g}",
                                      name=f"ct{g}")
                        nc.scalar.copy(ct[:], ps[0:P, :])
                        nc.vector.tensor_mul(dst, xsl, ct[:])
                    else:
                        nc.vector.tensor_mul(dst, xsl, ps[0:P, :])
                # capture DMA in slot-aligned batches (steps ..7 mod 8 end a
                # non-wrapping slot group; RD=16 gives 8 slots of slack)
                if i % 8 == 7 or i == NSTEP:
                    blo = last_dumped + 1
                    n = (i - blo + 1) * CG
                    for g in range(NG):
                        eng = nc.sync if (i // 8 + g) % 2 == 0 else nc.gpsimd
                        eng.dma_start(
                            qcap_dram[g][:, (blo - 1) * CG:(blo - 1) * CG + n],
                            rings[g][NCH * T:NCH * T + NCH,
                                     slot(blo):slot(blo) + n])
                    last_dumped = i

    nc.compile()
    return nc


def _get_program():
    if "p" not in _PROG_CACHE:
        _PROG_CACHE["p"] = _build_program()
    return _PROG_CACHE["p"]


def _host_prep(em, startt):
    """x = softmax over tags (start folded into step 0); a = log shifts."""
    x = em.astype(np.float32, copy=True)
    x[:, 0, :] += startt.astype(np.float32)
    mx = x.max(axis=2)
    x -= mx[:, :, None]
    np.exp(x, out=x)
    ssum = x.sum(axis=2)
    x /= ssum[:, :, None]
    x *= np.float32(2.0 ** CLOG2)
    a = mx.astype(np.float64) + np.log(ssum.astype(np.float64))
    return x, a


# chain mapping: chain_id = k*C + j  ->  seq b = id // M, segment s = id % M
_KJ = np.arange(NCH * C)
_SEQI = (_KJ // M).reshape(NCH, C)      # [k, j] -> local seq
_SEGI = (_KJ % M).reshape(NCH, C)       # [k, j] -> segment
_TSTART = np.array(T_START)
_STEPS = np.minimum(_TSTART[None, :] + np.arange(NSTEP + 1)[:, None], S - 1)


def _device_inputs(x, trans, endt):
    import ml_dtypes
    bf16 = ml_dtypes.bfloat16
    with np.errstate(under="ignore"):
        E = np.exp(trans.astype(np.float64)).astype(np.float32)
        wvec = np.exp(endt.astype(np.float64)).astype(np.float32)
    wcaug = np.zeros((NCH * T, PFULL), np.float32)
    for k in range(NCH):
        wcaug[k * T:(k + 1) * T, k * T:(k + 1) * T] = E
        wcaug[k * T:(k + 1) * T, NCH * T + k] = wvec
    wcaug = wcaug.astype(bf16)

    step_kj = _STEPS[:, _SEGI]           # [i, k, j] global step index
    in_maps = []
    for core in range(NCORES):
        xc = x[core * SEQ_PER_CORE:(core + 1) * SEQ_PER_CORE]  # [128, S, T]
        arr = xc[_SEQI[None, :, :], step_kj, :]   # [i, k, j, T]
        xseq = np.empty((P, (NSTEP + 1) * C), np.float32)
        xseq[0:NCH * T] = arr.transpose(1, 3, 0, 2).reshape(
            NCH * T, (NSTEP + 1) * C)
        xseq[NCH * T:] = 1.0
        in_maps.append({"xseq": xseq.astype(bf16), "wcaug": wcaug})
    return in_maps


def _denominators(res, big_a, tails):
    ln2 = np.log(2.0)
    spans = np.array(SPANS)
    den = np.zeros(B, np.float64)
    for core in range(NCORES):
        qg = [res.results[core][f"qcap{g}"].astype(np.float64)
              for g in range(NG)]                 # [3, NSTEP*CG] each
        q = np.concatenate(
            [a.reshape(NCH, NSTEP, CG) for a in qg], axis=2)  # [3,NSTEP,C]
        q = q.transpose(1, 0, 2).reshape(NSTEP, NCH * C)
        lq = (np.log(np.maximum(q, 1e-300))
              - CLOG2 * ln2 * (np.arange(NSTEP)[:, None] + 1.0))
        lq = lq.T.reshape(SEQ_PER_CORE, M, NSTEP)   # [b_local, s, j]
        bl = np.arange(SEQ_PER_CORE)
        gain = np.empty((SEQ_PER_CORE, M))
        gain[:, 0] = lq[:, 0, spans[0]]
        gain[:, 1:] = (np.take_along_axis(
            lq[:, 1:, :], (W + spans[1:])[None, :, None], axis=2)[:, :, 0]
            - lq[:, 1:, W])
        cum = np.concatenate([np.zeros((SEQ_PER_CORE, 1)),
                              np.cumsum(gain, axis=1)], axis=1)  # [b, s+1]
        tl = tails[core * SEQ_PER_CORE:(core + 1) * SEQ_PER_CORE]
        sstar = np.searchsorted(A_BOUND, tl, side="right") - 1
        li = tl - _TSTART[sstar]
        last = lq[bl, sstar, li] - np.where(sstar > 0, lq[bl, sstar, W], 0.0)
        den[core * SEQ_PER_CORE:(core + 1) * SEQ_PER_CORE] = (
            cum[bl, sstar] + last)
    return den + big_a[np.arange(B), tails]


def _numerator(em, tags, mask, startt, trans, endt):
    bsz, s_len, _ = em.shape
    tags = tags.astype(np.int64)
    ar = np.arange(s_len)
    bidx = np.arange(bsz)
    head = np.min(np.where(mask, ar[None, :], s_len - 1), axis=1)
    tail = np.max(ar[None, :] * mask, axis=1)
    nonempty = mask.sum(axis=1) != 0
    cond = mask[:, 1:] & (head[:, None] != ar[None, 1:])
    head_tags = tags[bidx, head]
    tail_tags = tags[bidx, tail]
    em64 = em.astype(np.float64)
    em_tag = np.take_along_axis(em64, tags[:, :, None], axis=2)[:, :, 0]
    trans_step = trans.astype(np.float64)[tags[:, :-1], tags[:, 1:]]
    num = (startt.astype(np.float64)[head_tags]
           + em_tag[bidx, head]
           + np.sum(np.where(cond, trans_step + em_tag[:, 1:], 0.0), axis=1)
           + endt.astype(np.float64)[tail_tags])
    return np.where(nonempty, num, 0.0)


def kernel(**inputs):
    from concourse.bass_utils import run_bass_kernel_spmd

    em = np.asarray(inputs["emissions"], dtype=np.float32)
    tags = np.asarray(inputs["tags"])
    mask = np.asarray(inputs["mask"]).astype(bool)
    startt = np.asarray(inputs["start_transitions"], dtype=np.float32)
    trans = np.asarray(inputs["transitions"], dtype=np.float32)
    endt = np.asarray(inputs["end_transitions"], dtype=np.float32)
    bsz, s_len, t = em.shape
    assert (bsz, s_len, t) == (B, S, T), (bsz, s_len, t)

    ar = np.arange(s_len)
    tails = np.max(ar[None, :] * mask, axis=1)
    nonempty = mask.sum(axis=1) != 0

    x, a = _host_prep(em, startt)
    big_a = np.cumsum(a, axis=1)
    nc = _get_program()
    in_maps = _device_inputs(x, trans, endt)
    res = run_bass_kernel_spmd(nc, in_maps, core_ids=list(range(NCORES)),
                               trace=TRACE)
    global LAST_RESULTS
    LAST_RESULTS = res

    den = np.where(nonempty, _denominators(res, big_a, tails), 0.0)
    num = _numerator(em, tags, mask, startt, trans, endt)
    llh = den - num
    labels = mask.sum(axis=1).astype(np.float64)
    eps = 1e-6
    out = np.sum(llh / (labels + eps)) / (np.sum(labels != 0) + eps)
    return np.asarray(out, dtype=np.float32)


# revision 7
# speedup vs baseline: 1.3660x; 1.3660x over previous
"""CRF token-mean NLL on 8 Trainium2 NeuronCores — time-segmented forward
algorithm with warmup-seeded parallel chains.

Math
----
out = sum_b(llh_b / labels_b) / count,  llh_b = den_b - num_b.
num (gold path score): host gather (numpy f64) — cheap, O(B*S).

den_b = logZ_b at tail_b via the forward algorithm in probability space:
x_j = softmax(em_j) (start folded into x_0), v_j = x_j . (E^T v_{j-1}),
logZ_j = log(w . v_j) + cumsum(a)_j, with a_j the softmax log-shifts and
w = exp(end).

Key idea: the recursion's DIRECTION forgets its initial condition at the
CRF mixing rate (a few steps here), only the log-MAGNITUDE accumulates.
So split the S=1024 steps into M=24 segments; each segment's chain is
seeded at (boundary - W) with the x-tile there and warmed up W=8 steps.
log Z telescopes through per-step captures q_j = w . v_j:
  logZ_tail = lq[0, span0] + sum_{0<s<s*} (lq[s, W+span_s] - lq[s, W])
              + lq[s*, tail-t_s*] - lq[s*, W] + big_a[tail]
(segment 0 is seeded exactly with v_0 = x_0, so its captures are
absolute).  Validated to rel err ~3e-8 vs f64.

Magnitude control: a constant 2^0.8125 per-step factor folded into the
host-prepped x stream keeps bf16 state magnitudes within 2^+-25 over the
51-step chains — no on-device renormalization at all.

Device structure (per core, 128 seqs x 24 segments = 3072 chains):
- State tiles [99, C]: 3 chain-blocks of 32 tags (partitions 0..95)
  + 3 capture rows (96..98).  Column c, block k = chain k*C+c.
- ONE stationary [96, 128] = blockdiag(E,E,E) plus capture columns
  (w replicated per block) padded to 128 for fast weight load; loaded
  effectively once.  Each matmul computes the state update AND
  q = w.v as 3 extra output partitions.
- Column groups (independent rings, staggered chains): per step-row and
  group, PE matmul -> elementwise x-multiply.  Group paths alternate:
  "act" = ScalarE psum->SBUF bf16 copy then VectorE 2x-mode multiply,
  "direct" = VectorE 1x multiply straight from PSUM — balancing ScalarE
  and VectorE load.  The x tiles carry ones in rows 96-98 so captures
  pass through the multiply into the state rings, DMA'd out in batches.
"""

import numpy as np

B, S, T = 1024, 1024, 32
NCORES = 8
SEQ_PER_CORE = 128
M = 24                      # time segments
W = 8                       # warmup steps per seeded chain
NCH = 3                     # chain blocks per tile
C = SEQ_PER_CORE * M // NCH  # 1024 columns
PATHS = ["direct", "direct"]  # per-group elementwise path
NG = len(PATHS)
CG = C // NG                # columns per group
P = NCH * T + NCH           # 99 live partitions (96 state + 3 capture)
PFULL = 128                 # padded matmul output partitions
RD = 16                     # ring depth (slots)
CHX = 4                     # x-tiles per streamed chunk
CLOG2 = 0.8125              # per-step 2^CLOG2 folded into x
A_BOUND = [round(S * s / M) for s in range(M + 1)]
SPANS = [A_BOUND[s + 1] - A_BOUND[s] for s in range(M)]
T_START = [0] + [A_BOUND[s] - W for s in range(1, M)]
NSTEP = max(SPANS) + W + 1  # 52 steps: captures q_0..q_{NSTEP-1}

_PROG_CACHE = {}
TRACE = False
LAST_RESULTS = None


def _build_program():
    import concourse.bacc as bacc
    import concourse.mybir as mybir
    from concourse import tile

    f32 = mybir.dt.float32
    bf16 = mybir.dt.bfloat16

    nc = bacc.Bacc("TRN2", target_bir_lowering=False, debug=False,
                   enable_asserts=False, num_devices=NCORES)

    # x stream: tile i at cols [i*C, (i+1)*C); tile 0 is the seed
    xseq_dram = nc.dram_tensor("xseq", [P, (NSTEP + 1) * C], bf16,
                               kind="ExternalInput")
    wcaug_dram = nc.dram_tensor("wcaug", [NCH * T, PFULL], bf16,
                                kind="ExternalInput")
    qcap_dram = [nc.dram_tensor(f"qcap{g}", [NCH, NSTEP * CG], bf16,
                                kind="ExternalOutput") for g in range(NG)]

    nchunks = (NSTEP + CHX - 1) // CHX  # chunks cover tiles 1..NSTEP

    with tile.TileContext(nc) as tc:
        with (
            tc.tile_pool(name="const", bufs=1) as constp,
            tc.tile_pool(name="ringp", bufs=1) as ringp,
            tc.tile_pool(name="xs", bufs=3) as xp,
            tc.tile_pool(name="ct", bufs=4) as ctp,
            tc.tile_pool(name="ps", bufs=4, space="PSUM") as psp,
        ):
            wcaug = constp.tile([NCH * T, PFULL], bf16)
            nc.sync.dma_start(wcaug[:], wcaug_dram[:])

            rings = [ringp.tile([P, RD * CG], bf16, name=f"ring{g}")
                     for g in range(NG)]
            for g in range(NG):
                nc.gpsimd.dma_start(rings[g][:, 0:CG],
                                    xseq_dram[:, g * CG:(g + 1) * CG])

            xtiles = {}

            def ensure_chunk(cid):
                if cid in xtiles or cid >= nchunks:
                    return
                lo = (1 + cid * CHX) * C
                n = min(CHX * C, (NSTEP + 1) * C - lo)
                tf = xp.tile([P, CHX * C], bf16, tag="xc", name="xc")
                half = n // 2
                eng = nc.sync if cid % 2 == 0 else nc.gpsimd
                eng.dma_start(tf[:, 0:half], xseq_dram[:, lo:lo + half])
                eng.dma_start(tf[:, half:n], xseq_dram[:, lo + half:lo + n])
                xtiles[cid] = tf

            ensure_chunk(0)
            ensure_chunk(1)
            ensure_chunk(2)

            def slot(i):
                return (i % RD) * CG

            last_dumped = 0
            for i in range(1, NSTEP + 1):
                cid = (i - 1) // CHX
                off = ((i - 1) % CHX) * C
                if (i - 1) % CHX == 0:
                    ensure_chunk(cid + 3)
                xt = xtiles[cid]
                for g in range(NG):
                    ring = rings[g]
                    xsl = xt[:, off + g * CG:off + (g + 1) * CG]
                    ps = psp.tile([PFULL, CG], f32, tag=f"ps{g}",
                                  name=f"ps{g}")
                    nc.tensor.matmul(
                        ps[:], wcaug[:],
                        ring[0:NCH * T, slot(i - 1):slot(i - 1) + CG])
                    dst = ring[:, slot(i):slot(i) + CG]
                    if PATHS[g] == "act":
                        ct = ctp.tile([P, CG], bf16, tag=f"ct{g}",
                                      name=f"ct{g}")
                        nc.scalar.copy(ct[:], ps[0:P, :])
                        nc.vector.tensor_mul(dst, xsl, ct[:])
                    else:
                        nc.vector.tensor_mul(dst, xsl, ps[0:P, :])
                # capture DMA in slot-aligned batches (steps ..7 mod 8 end a
                # non-wrapping slot group; RD=16 gives 8 slots of slack)
                if i % 8 == 7 or i == NSTEP:
                    blo = last_dumped + 1
                    n = (i - blo + 1) * CG
                    for g in range(NG):
                        eng = nc.sync if (i // 8 + g) % 2 == 0 else nc.gpsimd
                        eng.dma_start(
                            qcap_dram[g][:, (blo - 1) * CG:(blo - 1) * CG + n],
                            rings[g][NCH * T:NCH * T + NCH,
                                     slot(blo):slot(blo) + n])
                    last_dumped = i

    nc.compile()
    return nc


def _get_program():
    if "p" not in _PROG_CACHE:
        _PROG_CACHE["p"] = _build_program()
    return _PROG_CACHE["p"]


def _host_prep(em, startt):
    """x = softmax over tags (start folded into step 0); a = log shifts."""
    x = em.astype(np.float32, copy=True)
    x[:, 0, :] += startt.astype(np.float32)
    mx = x.max(axis=2)
    x -= mx[:, :, None]
    np.exp(x, out=x)
    ssum = x.sum(axis=2)
    x /= ssum[:, :, None]
    x *= np.float32(2.0 ** CLOG2)
    a = mx.astype(np.float64) + np.log(ssum.astype(np.float64))
    return x, a


# chain mapping: chain_id = k*C + j  ->  seq b = id // M, segment s = id % M
_KJ = np.arange(NCH * C)
_SEQI = (_KJ // M).reshape(NCH, C)      # [k, j] -> local seq
_SEGI = (_KJ % M).reshape(NCH, C)       # [k, j] -> segment
_TSTART = np.array(T_START)
_STEPS = np.minimum(_TSTART[None, :] + np.arange(NSTEP + 1)[:, None], S - 1)


def _device_inputs(x, trans, endt):
    import ml_dtypes
    bf16 = ml_dtypes.bfloat16
    with np.errstate(under="ignore"):
        E = np.exp(trans.astype(np.float64)).astype(np.float32)
        wvec = np.exp(endt.astype(np.float64)).astype(np.float32)
    wcaug = np.zeros((NCH * T, PFULL), np.float32)
    for k in range(NCH):
        wcaug[k * T:(k + 1) * T, k * T:(k + 1) * T] = E
        wcaug[k * T:(k + 1) * T, NCH * T + k] = wvec
    wcaug = wcaug.astype(bf16)

    step_kj = _STEPS[:, _SEGI]           # [i, k, j] global step index
    in_maps = []
    for core in range(NCORES):
        xc = x[core * SEQ_PER_CORE:(core + 1) * SEQ_PER_CORE]  # [128, S, T]
        arr = xc[_SEQI[None, :, :], step_kj, :]   # [i, k, j, T]
        xseq = np.empty((P, (NSTEP + 1) * C), np.float32)
        xseq[0:NCH * T] = arr.transpose(1, 3, 0, 2).reshape(
            NCH * T, (NSTEP + 1) * C)
        xseq[NCH * T:] = 1.0
        in_maps.append({"xseq": xseq.astype(bf16), "wcaug": wcaug})
    return in_maps


def _denominators(res, big_a, tails):
    ln2 = np.log(2.0)
    spans = np.array(SPANS)
    den = np.zeros(B, np.float64)
    for core in range(NCORES):
        qg = [res.results[core][f"qcap{g}"].astype(np.float64)
              for g in range(NG)]                 # [3, NSTEP*CG] each
        q = np.concatenate(
            [a.reshape(NCH, NSTEP, CG) for a in qg], axis=2)  # [3,NSTEP,C]
        q = q.transpose(1, 0, 2).reshape(NSTEP, NCH * C)
        lq = (np.log(np.maximum(q, 1e-300))
              - CLOG2 * ln2 * (np.arange(NSTEP)[:, None] + 1.0))
        lq = lq.T.reshape(SEQ_PER_CORE, M, NSTEP)   # [b_local, s, j]
        bl = np.arange(SEQ_PER_CORE)
        gain = np.empty((SEQ_PER_CORE, M))
        gain[:, 0] = lq[:, 0, spans[0]]
        gain[:, 1:] = (np.take_along_axis(
            lq[:, 1:, :], (W + spans[1:])[None, :, None], axis=2)[:, :, 0]
            - lq[:, 1:, W])
        cum = np.concatenate([np.zeros((SEQ_PER_CORE, 1)),
                              np.cumsum(gain, axis=1)], axis=1)  # [b, s+1]
        tl = tails[core * SEQ_PER_CORE:(core + 1) * SEQ_PER_CORE]
        sstar = np.searchsorted(A_BOUND, tl, side="right") - 1
        li = tl - _TSTART[sstar]
        last = lq[bl, sstar, li] - np.where(sstar > 0, lq[bl, sstar, W], 0.0)
        den[core * SEQ_PER_CORE:(core + 1) * SEQ_PER_CORE] = (
            cum[bl, sstar] + last)
    return den + big_a[np.arange(B), tails]


def _numerator(em, tags, mask, startt, trans, endt):
    bsz, s_len, _ = em.shape
    tags = tags.astype(np.int64)
    ar = np.arange(s_len)
    bidx = np.arange(bsz)
    head = np.min(np.where(mask, ar[None, :], s_len - 1), axis=1)
    tail = np.max(ar[None, :] * mask, axis=1)
    nonempty = mask.sum(axis=1) != 0
    cond = mask[:, 1:] & (head[:, None] != ar[None, 1:])
    head_tags = tags[bidx, head]
    tail_tags = tags[bidx, tail]
    em64 = em.astype(np.float64)
    em_tag = np.take_along_axis(em64, tags[:, :, None], axis=2)[:, :, 0]
    trans_step = trans.astype(np.float64)[tags[:, :-1], tags[:, 1:]]
    num = (startt.astype(np.float64)[head_tags]
           + em_tag[bidx, head]
           + np.sum(np.where(cond, trans_step + em_tag[:, 1:], 0.0), axis=1)
           + endt.astype(np.float64)[tail_tags])
    return np.where(nonempty, num, 0.0)


def kernel(**inputs):
    from concourse.bass_utils import run_bass_kernel_spmd

    em = np.asarray(inputs["emissions"], dtype=np.float32)
    tags = np.asarray(inputs["tags"])
    mask = np.asarray(inputs["mask"]).astype(bool)
    startt = np.asarray(inputs["start_transitions"], dtype=np.float32)
    trans = np.asarray(inputs["transitions"], dtype=np.float32)
    endt = np.asarray(inputs["end_transitions"], dtype=np.float32)
    bsz, s_len, t = em.shape
    assert (bsz, s_len, t) == (B, S, T), (bsz, s_len, t)

    ar = np.arange(s_len)
    tails = np.max(ar[None, :] * mask, axis=1)
    nonempty = mask.sum(axis=1) != 0

    x, a = _host_prep(em, startt)
    big_a = np.cumsum(a, axis=1)
    nc = _get_program()
    in_maps = _device_inputs(x, trans, endt)
    res = run_bass_kernel_spmd(nc, in_maps, core_ids=list(range(NCORES)),
                               trace=TRACE)
    global LAST_RESULTS
    LAST_RESULTS = res

    den = np.where(nonempty, _denominators(res, big_a, tails), 0.0)
    num = _numerator(em, tags, mask, startt, trans, endt)
    llh = den - num
    labels = mask.sum(axis=1).astype(np.float64)
    eps = 1e-6
    out = np.sum(llh / (labels + eps)) / (np.sum(labels != 0) + eps)
    return np.asarray(out, dtype=np.float32)


# revision 10
# speedup vs baseline: 1.4886x; 1.0897x over previous
"""CRF token-mean NLL on 8 Trainium2 NeuronCores — time-segmented forward
algorithm with warmup-seeded parallel chains.

Math
----
out = sum_b(llh_b / labels_b) / count,  llh_b = den_b - num_b.
num (gold path score): host gather (numpy f64) — cheap, O(B*S).

den_b = logZ_b at tail_b via the forward algorithm in probability space:
x_j = softmax(em_j) (start folded into x_0), v_j = x_j . (E^T v_{j-1}),
logZ_j = log(w . v_j) + cumsum(a)_j, with a_j the softmax log-shifts and
w = exp(end).

Key idea: the recursion's DIRECTION forgets its initial condition at the
CRF mixing rate (a few steps here), only the log-MAGNITUDE accumulates.
So split the S=1024 steps into M=24 segments; each segment's chain is
seeded at (boundary - W) with the x-tile there and warmed up W=8 steps.
log Z telescopes through per-step captures q_j = w . v_j:
  logZ_tail = lq[0, span0] + sum_{0<s<s*} (lq[s, W+span_s] - lq[s, W])
              + lq[s*, tail-t_s*] - lq[s*, W] + big_a[tail]
(segment 0 is seeded exactly with v_0 = x_0, so its captures are
absolute).  Validated to rel err ~3e-8 vs f64.

Magnitude control: a constant 2^0.8125 per-step factor folded into the
host-prepped x stream keeps bf16 state magnitudes within 2^+-25 over the
51-step chains — no on-device renormalization at all.

Device structure (per core, 128 seqs x 24 segments = 3072 chains):
- State tiles [99, C]: 3 chain-blocks of 32 tags (partitions 0..95)
  + 3 capture rows (96..98).  Column c, block k = chain k*C+c.
- ONE stationary [96, 128] = blockdiag(E,E,E) plus capture columns
  (w replicated per block) padded to 128 for fast weight load; loaded
  effectively once.  Each matmul computes the state update AND
  q = w.v as 3 extra output partitions.
- Column groups (independent rings, staggered chains): per step-row and
  group, PE matmul -> elementwise x-multiply.  Group paths alternate:
  "act" = ScalarE psum->SBUF bf16 copy then VectorE 2x-mode multiply,
  "direct" = VectorE 1x multiply straight from PSUM — balancing ScalarE
  and VectorE load.  The x tiles carry ones in rows 96-98 so captures
  pass through the multiply into the state rings, DMA'd out in batches.
"""

import numpy as np

B, S, T = 1024, 1024, 32
NCORES = 8
SEQ_PER_CORE = 128
M = 24                      # time segments
W = 4                       # warmup steps per seeded chain
NCH = 3                     # chain blocks per tile
C = SEQ_PER_CORE * M // NCH  # 1024 columns
PATHS = ["direct", "direct"]  # per-group elementwise path
NG = len(PATHS)
CG = C // NG                # columns per group
P = NCH * T + NCH           # 99 live partitions (96 state + 3 capture)
PFULL = 128                 # padded matmul output partitions
RD = 16                     # ring depth (slots)
CHX = 4                     # x-tiles per streamed chunk
CLOG2 = 0.8125              # per-step 2^CLOG2 folded into x
A_BOUND = [round(S * s / M) for s in range(M + 1)]
SPANS = [A_BOUND[s + 1] - A_BOUND[s] for s in range(M)]
T_START = [0] + [A_BOUND[s] - W for s in range(1, M)]
NSTEP = max(SPANS) + W + 1  # 52 steps: captures q_0..q_{NSTEP-1}

_PROG_CACHE = {}
TRACE = False
LAST_RESULTS = None


def _build_program():
    import concourse.bacc as bacc
    import concourse.mybir as mybir
    from concourse import tile

    f32 = mybir.dt.float32
    bf16 = mybir.dt.bfloat16

    nc = bacc.Bacc("TRN2", target_bir_lowering=False, debug=False,
                   enable_asserts=False, num_devices=NCORES)

    # x stream: tile i at cols [i*C, (i+1)*C); tile 0 is the seed
    xseq_dram = nc.dram_tensor("xseq", [P, (NSTEP + 1) * C], bf16,
                               kind="ExternalInput")
    wcaug_dram = nc.dram_tensor("wcaug", [NCH * T, PFULL], bf16,
                                kind="ExternalInput")
    qcap_dram = [nc.dram_tensor(f"qcap{g}", [NCH, NSTEP * CG], bf16,
                                kind="ExternalOutput") for g in range(NG)]

    nchunks = (NSTEP + CHX - 1) // CHX  # chunks cover tiles 1..NSTEP

    with tile.TileContext(nc) as tc:
        with (
            tc.tile_pool(name="const", bufs=1) as constp,
            tc.tile_pool(name="ringp", bufs=1) as ringp,
            tc.tile_pool(name="xs", bufs=3) as xp,
            tc.tile_pool(name="ct", bufs=4) as ctp,
            tc.tile_pool(name="ps", bufs=4, space="PSUM") as psp,
        ):
            wcaug = constp.tile([NCH * T, PFULL], bf16)
            nc.sync.dma_start(wcaug[:], wcaug_dram[:])

            rings = [ringp.tile([P, RD * CG], bf16, name=f"ring{g}")
                     for g in range(NG)]
            nc.gpsimd.dma_start(rings[0][:, 0:CG], xseq_dram[:, 0:CG])
            nc.scalar.dma_start(rings[1][:, 0:CG], xseq_dram[:, CG:2 * CG])

            xtiles = {}
            dmaq = [nc.sync, nc.gpsimd, nc.scalar]

            def ensure_chunk(cid, parts=2):
                if cid in xtiles or cid >= nchunks:
                    return
                lo = (1 + cid * CHX) * C
                n = min(CHX * C, (NSTEP + 1) * C - lo)
                tf = xp.tile([P, CHX * C], bf16, tag="xc", name="xc")
                step = (n + parts - 1) // parts
                for p in range(parts):
                    a, b = p * step, min((p + 1) * step, n)
                    dmaq[(cid + p) % 3].dma_start(
                        tf[:, a:b], xseq_dram[:, lo + a:lo + b])
                xtiles[cid] = tf

            ensure_chunk(0, parts=4)
            ensure_chunk(1)
            ensure_chunk(2)

            def slot(i):
                return (i % RD) * CG

            last_dumped = 0
            for i in range(1, NSTEP + 1):
                cid = (i - 1) // CHX
                off = ((i - 1) % CHX) * C
                if (i - 1) % CHX == 0:
                    ensure_chunk(cid + 3)
                xt = xtiles[cid]
                for g in range(NG):
                    ring = rings[g]
                    xsl = xt[:, off + g * CG:off + (g + 1) * CG]
                    ps = psp.tile([PFULL, CG], f32, tag=f"ps{g}",
                                  name=f"ps{g}")
                    nc.tensor.matmul(
                        ps[:], wcaug[:],
                        ring[0:NCH * T, slot(i - 1):slot(i - 1) + CG])
                    dst = ring[:, slot(i):slot(i) + CG]
                    if PATHS[g] == "act":
                        ct = ctp.tile([P, CG], bf16, tag=f"ct{g}",
                                      name=f"ct{g}")
                        nc.scalar.copy(ct[:], ps[0:P, :])
                        nc.vector.tensor_mul(dst, xsl, ct[:])
                    else:
                        nc.vector.tensor_mul(dst, xsl, ps[0:P, :])
                # capture DMA in slot-aligned batches (steps ..7 mod 8 end a
                # non-wrapping slot group; RD=16 gives 8 slots of slack)
                if i % 8 == 7 or i == NSTEP:
                    blo = last_dumped + 1
                    n = (i - blo + 1) * CG
                    for g in range(NG):
                        eng = dmaq[(i // 8 + g) % 3]
                        eng.dma_start(
                            qcap_dram[g][:, (blo - 1) * CG:(blo - 1) * CG + n],
                            rings[g][NCH * T:NCH * T + NCH,
                                     slot(blo):slot(blo) + n])
                    last_dumped = i

    nc.compile()
    return nc


def _get_program():
    if "p" not in _PROG_CACHE:
        _PROG_CACHE["p"] = _build_program()
    return _PROG_CACHE["p"]


def _host_prep(em, startt):
    """x = softmax over tags (start folded into step 0); a = log shifts."""
    x = em.astype(np.float32, copy=True)
    x[:, 0, :] += startt.astype(np.float32)
    mx = x.max(axis=2)
    x -= mx[:, :, None]
    np.exp(x, out=x)
    ssum = x.sum(axis=2)
    x /= ssum[:, :, None]
    x *= np.float32(2.0 ** CLOG2)
    a = mx.astype(np.float64) + np.log(ssum.astype(np.float64))
    return x, a


# chain mapping: chain_id = k*C + j  ->  seq b = id // M, segment s = id % M
_KJ = np.arange(NCH * C)
_SEQI = (_KJ // M).reshape(NCH, C)      # [k, j] -> local seq
_SEGI = (_KJ % M).reshape(NCH, C)       # [k, j] -> segment
_TSTART = np.array(T_START)
_STEPS = np.minimum(_TSTART[None, :] + np.arange(NSTEP + 1)[:, None], S - 1)


def _device_inputs(x, trans, endt):
    import ml_dtypes
    bf16 = ml_dtypes.bfloat16
    with np.errstate(under="ignore"):
        E = np.exp(trans.astype(np.float64)).astype(np.float32)
        wvec = np.exp(endt.astype(np.float64)).astype(np.float32)
    wcaug = np.zeros((NCH * T, PFULL), np.float32)
    for k in range(NCH):
        wcaug[k * T:(k + 1) * T, k * T:(k + 1) * T] = E
        wcaug[k * T:(k + 1) * T, NCH * T + k] = wvec
    wcaug = wcaug.astype(bf16)

    step_kj = _STEPS[:, _SEGI]           # [i, k, j] global step index
    in_maps = []
    for core in range(NCORES):
        xc = x[core * SEQ_PER_CORE:(core + 1) * SEQ_PER_CORE]  # [128, S, T]
        arr = xc[_SEQI[None, :, :], step_kj, :]   # [i, k, j, T]
        xseq = np.empty((P, (NSTEP + 1) * C), np.float32)
        xseq[0:NCH * T] = arr.transpose(1, 3, 0, 2).reshape(
            NCH * T, (NSTEP + 1) * C)
        xseq[NCH * T:] = 1.0
        in_maps.append({"xseq": xseq.astype(bf16), "wcaug": wcaug})
    return in_maps


def _denominators(res, big_a, tails):
    ln2 = np.log(2.0)
    spans = np.array(SPANS)
    den = np.zeros(B, np.float64)
    for core in range(NCORES):
        qg = [res.results[core][f"qcap{g}"].astype(np.float64)
              for g in range(NG)]                 # [3, NSTEP*CG] each
        q = np.concatenate(
            [a.reshape(NCH, NSTEP, CG) for a in qg], axis=2)  # [3,NSTEP,C]
        q = q.transpose(1, 0, 2).reshape(NSTEP, NCH * C)
        lq = (np.log(np.maximum(q, 1e-300))
              - CLOG2 * ln2 * (np.arange(NSTEP)[:, None] + 1.0))
        lq = lq.T.reshape(SEQ_PER_CORE, M, NSTEP)   # [b_local, s, j]
        bl = np.arange(SEQ_PER_CORE)
        gain = np.empty((SEQ_PER_CORE, M))
        gain[:, 0] = lq[:, 0, spans[0]]
        gain[:, 1:] = (np.take_along_axis(
            lq[:, 1:, :], (W + spans[1:])[None, :, None], axis=2)[:, :, 0]
            - lq[:, 1:, W])
        cum = np.concatenate([np.zeros((SEQ_PER_CORE, 1)),
                              np.cumsum(gain, axis=1)], axis=1)  # [b, s+1]
        tl = tails[core * SEQ_PER_CORE:(core + 1) * SEQ_PER_CORE]
        sstar = np.searchsorted(A_BOUND, tl, side="right") - 1
        li = tl - _TSTART[sstar]
        last = lq[bl, sstar, li] - np.where(sstar > 0, lq[bl, sstar, W], 0.0)
        den[core * SEQ_PER_CORE:(core + 1) * SEQ_PER_CORE] = (
            cum[bl, sstar] + last)
    return den + big_a[np.arange(B), tails]


def _numerator(em, tags, mask, startt, trans, endt):
    bsz, s_len, _ = em.shape
    tags = tags.astype(np.int64)
    ar = np.arange(s_len)
    bidx = np.arange(bsz)
    head = np.min(np.where(mask, ar[None, :], s_len - 1), axis=1)
    tail = np.max(ar[None, :] * mask, axis=1)
    nonempty = mask.sum(axis=1) != 0
    cond = mask[:, 1:] & (head[:, None] != ar[None, 1:])
    head_tags = tags[bidx, head]
    tail_tags = tags[bidx, tail]
    em64 = em.astype(np.float64)
    em_tag = np.take_along_axis(em64, tags[:, :, None], axis=2)[:, :, 0]
    trans_step = trans.astype(np.float64)[tags[:, :-1], tags[:, 1:]]
    num = (startt.astype(np.float64)[head_tags]
           + em_tag[bidx, head]
           + np.sum(np.where(cond, trans_step + em_tag[:, 1:], 0.0), axis=1)
           + endt.astype(np.float64)[tail_tags])
    return np.where(nonempty, num, 0.0)


def kernel(**inputs):
    from concourse.bass_utils import run_bass_kernel_spmd

    em = np.asarray(inputs["emissions"], dtype=np.float32)
    tags = np.asarray(inputs["tags"])
    mask = np.asarray(inputs["mask"]).astype(bool)
    startt = np.asarray(inputs["start_transitions"], dtype=np.float32)
    trans = np.asarray(inputs["transitions"], dtype=np.float32)
    endt = np.asarray(inputs["end_transitions"], dtype=np.float32)
    bsz, s_len, t = em.shape
    assert (bsz, s_len, t) == (B, S, T), (bsz, s_len, t)

    ar = np.arange(s_len)
    tails = np.max(ar[None, :] * mask, axis=1)
    nonempty = mask.sum(axis=1) != 0

    x, a = _host_prep(em, startt)
    big_a = np.cumsum(a, axis=1)
    nc = _get_program()
    in_maps = _device_inputs(x, trans, endt)
    res = run_bass_kernel_spmd(nc, in_maps, core_ids=list(range(NCORES)),
                               trace=TRACE)
    global LAST_RESULTS
    LAST_RESULTS = res

    den = np.where(nonempty, _denominators(res, big_a, tails), 0.0)
    num = _numerator(em, tags, mask, startt, trans, endt)
    llh = den - num
    labels = mask.sum(axis=1).astype(np.float64)
    eps = 1e-6
    out = np.sum(llh / (labels + eps)) / (np.sum(labels != 0) + eps)
    return np.asarray(out, dtype=np.float32)
